# revision 13
# baseline (speedup 1.0000x reference)
"""DigitCaps (CapsNet dynamic routing) Trainium2 kernel.

Math (per reference):
  u_hat[b,i,o,d] = sum_k W[i,o,d,k] * x[b,i,k]      B=256, IC=1152, K=8, O=10, D=16
  3 routing iters: c = softmax_o(bl); s = sum_i c*u_hat; v = squash(s);
                   bl += sum_d u_hat*v
  out v: [B, 10, 16]

Strategy: pure data-parallel over batch, 8 cores x 32 samples. Per core,
batch processed in 4 groups of 8 ("bgroups"). The einsum runs on the
TensorEngine using a host-built block-diagonal x operand:
  lhsT = xblk[g] [(i16,k8)=128, (i16,b8)=128]  (block-diag, stationary)
  rhs  = wr[g]   [(i16,k8)=128, (o,d)=160]
  out  = u_hat   [(i16,b8)=128, (o,d)=160] per group g of 16 in-caps.
u_hat stays on-chip (bf16, f32 accum in PSUM). Routing:
  - s-matmul: lhsT = C[g] [(i16,b)=128, (b',o)=80] (c with delta_{bb'} mask,
    built by one broadcast-multiply), rhs = u_hat[g], PSUM-accum over g.
  - diag(o=o') extracted via mask-multiply + reduce.
  - bl-update on VectorE: z = u_hat * vrep (broadcast), tree-reduce over d.
All engine ops emitted under TileContext (auto scheduling/semaphores).
"""

import sys

sys.path.insert(0, "/opt/trn_rl_repo")

import numpy as np
import ml_dtypes

import concourse.bass as bass
import concourse.bacc as bacc_mod
from concourse import mybir
from concourse.tile import TileContext
from concourse.bass_utils import run_bass_kernel_spmd

BF16 = ml_dtypes.bfloat16

# Problem dims (hardcoded per harness contract)
B, IC, KD, OC, OD = 256, 1152, 8, 10, 16
NCORES = 8
BL = B // NCORES          # 32 samples per core
BG = 8                    # bgroup size
NBG = BL // BG            # 4 bgroups
G = IC // 16              # 72 groups of 16 in-caps
ODF = OC * OD             # 160
ITERS = 3
GO = G * OC               # 720 logit columns
ZCH = 18                  # g-chunk size for the bl-update pipeline
NZCH = G // ZCH           # 4 chunks

_BUILT = None  # cached (nc)
LEVEL = 5  # feature bisection: 1=einsum 2=+s/squash 3=+vrep 4=+bl-update 5=full


def _consts():
    """Host-side constant tensors shared by all cores."""
    p = np.arange(128)
    bb_of_p = p % 8  # b-lane of partition (i_sub,b)

    # mcb [128, 80] bf16: delta(b(p) == b') at column (b'*10+o)
    col_b = (np.arange(80) // 10)
    mcb = (bb_of_p[:, None] == col_b[None, :]).astype(np.float32)

    # c0t [128, 80] bf16: 0.1 * same delta (uniform coupling, iter 0)
    c0t = 0.1 * mcb

    # msks [80, 160] f32: delta(o == o') ; row (b,o), col (o'*16+d)
    row_o = np.arange(80) % 10
    col_o = np.arange(160) // 16
    msks = (row_o[:, None] == col_o[None, :]).astype(np.float32)

    # arep [80, 128] bf16: delta(b == b') ; row (b,o), col (i_sub*8+b')
    row_b = np.arange(80) // 10
    col_b2 = np.arange(128) % 8
    arep = (row_b[:, None] == col_b2[None, :]).astype(np.float32)

    return (
        mcb.astype(BF16),
        c0t.astype(BF16),
        msks,
        arep.astype(BF16),
    )


def _prep_core(x_c, W_r):
    """Per-core input prep. x_c: [32, 1152, 8] f32. W_r: host-prepped wr
    [128, G*160] bf16 (shared). Returns dict of DRAM inputs."""
    # xblk [NBG, 128, G*128] bf16 block-diagonal:
    #   xblk[bg, i_sub*8+k, g*128 + i_sub*8+b] = x_c[bg*8+b, g*16+i_sub, k]
    xblk = np.zeros((NBG, 128, G * 128), np.float32)
    xv = x_c.reshape(NBG, BG, G, 16, KD)  # [bg, b, g, i_sub, k]
    # scatter diagonal blocks
    for i_sub in range(16):
        # rows i_sub*8 + k (k=0..7); cols g*128 + i_sub*8 + b
        blk = xv[:, :, :, i_sub, :].transpose(0, 3, 2, 1)  # [bg, k, g, b]
        xblk[:, i_sub * 8 : i_sub * 8 + 8, :].reshape(NBG, 8, G, 128)[
            :, :, :, i_sub * 8 : i_sub * 8 + 8
        ] = blk
    return {"xblk": xblk.astype(BF16)}


def _prep_w(W):
    """wr [128, G*160] bf16: wr[i_sub*8+k, g*160 + o*16+d] = W[g*16+i_sub,o,d,k]"""
    wv = W.reshape(G, 16, OC, OD, KD)  # [g, i_sub, o, d, k]
    wr = wv.transpose(1, 4, 0, 2, 3).reshape(128, G * ODF)
    return np.ascontiguousarray(wr).astype(BF16)


def _build():
    global _BUILT
    if _BUILT is not None:
        return _BUILT

    nc = bacc_mod.Bacc()
    dt = mybir.dt
    xblk_d = nc.dram_tensor("xblk", [NBG, 128, G * 128], dt.bfloat16, kind="ExternalInput")
    wr_d = nc.dram_tensor("wr", [128, G * ODF], dt.bfloat16, kind="ExternalInput")
    mcb_d = nc.dram_tensor("mcb", [128, 80], dt.bfloat16, kind="ExternalInput")
    c0t_d = nc.dram_tensor("c0t", [128, 80], dt.bfloat16, kind="ExternalInput")
    msks_d = nc.dram_tensor("msks", [80, ODF], dt.float32, kind="ExternalInput")
    arep_d = nc.dram_tensor("arep", [80, 128], dt.bfloat16, kind="ExternalInput")
    vout_d = nc.dram_tensor("vout", [BL, OC, OD], dt.float32, kind="ExternalOutput")

    AF = mybir.ActivationFunctionType
    ALU = mybir.AluOpType
    AX = mybir.AxisListType

    with TileContext(nc) as tc:
        with (
            tc.tile_pool(name="consts", bufs=1) as cpool,
            tc.tile_pool(name="wrp", bufs=1) as wpool,
            tc.tile_pool(name="xbp", bufs=2) as xpool,
            tc.tile_pool(name="uhp", bufs=2) as uhpool,
            tc.tile_pool(name="route", bufs=2) as rpool,
            tc.tile_pool(name="ztmp", bufs=2) as zpool,
            tc.tile_pool(name="small", bufs=3) as spool,
            tc.tile_pool(name="pe", bufs=4, space="PSUM") as pe_pool,
            tc.tile_pool(name="ps", bufs=2, space="PSUM") as ps_pool,
            tc.tile_pool(name="pv", bufs=2, space="PSUM") as pv_pool,
        ):
            # resident constants
            wr_sb = wpool.tile([128, G * ODF], dt.bfloat16, tag="wr")
            for gt in range(G // 3):
                nc.sync.dma_start(
                    out=wr_sb[:, gt * 3 * ODF : (gt + 1) * 3 * ODF],
                    in_=wr_d[:, gt * 3 * ODF : (gt + 1) * 3 * ODF],
                )
            mcb = cpool.tile([128, 80], dt.bfloat16, tag="mcb")
            nc.sync.dma_start(out=mcb[:], in_=mcb_d[:])
            c0t = cpool.tile([128, 80], dt.bfloat16, tag="c0t")
            nc.sync.dma_start(out=c0t[:], in_=c0t_d[:])
            msks = cpool.tile([80, ODF], dt.float32, tag="msks")
            nc.sync.dma_start(out=msks[:], in_=msks_d[:])
            arep = cpool.tile([80, 128], dt.bfloat16, tag="arep")
            nc.sync.dma_start(out=arep[:], in_=arep_d[:])
            czero = cpool.tile([128, 1], dt.float32, tag="czero")
            nc.vector.memset(czero[:], 0.0)
            ceps = cpool.tile([80, 1], dt.float32, tag="ceps")
            nc.vector.memset(ceps[:], 1e-8)

            for bg in range(NBG):
                # ---- load block-diag x for this bgroup
                xblk = xpool.tile([128, G * 128], dt.bfloat16, tag="xblk")
                for gt in range(G // 3):
                    nc.sync.dma_start(
                        out=xblk[:, gt * 384 : (gt + 1) * 384],
                        in_=xblk_d[bg][:, gt * 384 : (gt + 1) * 384],
                    )

                # ---- einsum: u_hat tiles, 3 groups per PSUM bank
                uh = uhpool.tile([128, G * ODF], dt.bfloat16, tag="uh")
                for gt in range(G // 3):
                    pe = pe_pool.tile([128, 480], dt.float32, tag="pe")
                    for j in range(3):
                        g = gt * 3 + j
                        nc.tensor.matmul(
                            pe[:, j * ODF : (j + 1) * ODF],
                            xblk[:, g * 128 : (g + 1) * 128],
                            wr_sb[:, g * ODF : (g + 1) * ODF],
                            start=True,
                            stop=True,
                        )
                    # evict-cast to bf16 on ScalarE
                    nc.scalar.copy(
                        out=uh[:, gt * 480 : (gt + 1) * 480], in_=pe[:]
                    )

                # ---- routing state
                bl = rpool.tile([128, GO], dt.float32, tag="bl")
                nc.vector.memset(bl[:], 0.0)
                ee = rpool.tile([128, GO], dt.float32, tag="ee")
                zz = rpool.tile([128, G], dt.float32, tag="zz")
                rz = rpool.tile([128, G], dt.float32, tag="rz")
                cC = rpool.tile([128, GO], dt.float32, tag="cC")
                cCb = rpool.tile([128, GO], dt.bfloat16, tag="cCb")
                call = rpool.tile([128, G * 80], dt.bfloat16, tag="call")

                v_bf = None
                if LEVEL < 2:
                    dbg = spool.tile([80, OD], dt.float32, tag="dbg")
                    nc.vector.tensor_copy(out=dbg[:], in_=uh[:80, :OD])
                    nc.sync.dma_start(
                        out=vout_d[bg * BG : (bg + 1) * BG].rearrange(
                            "b o d -> (b o) d"
                        ),
                        in_=dbg[:],
                    )
                    continue
                n_it = ITERS if LEVEL >= 5 else 1
                for it in range(n_it):
                    if it > 0 and LEVEL >= 5:
                        # c = softmax_o(bl), then mask to block-diag layout
                        nc.scalar.activation(out=ee[:], in_=bl[:], func=AF.Exp, bias=czero[:])
                        nc.vector.tensor_reduce(
                            out=zz[:],
                            in_=ee[:].rearrange("p (g o) -> p g o", o=OC),
                            axis=AX.X,
                            op=ALU.add,
                        )
                        nc.vector.reciprocal(out=rz[:], in_=zz[:])
                        nc.vector.tensor_tensor(
                            out=cC[:].rearrange("p (g o) -> p g o", o=OC),
                            in0=ee[:].rearrange("p (g o) -> p g o", o=OC),
                            in1=rz[:].unsqueeze(2).broadcast_to(
                                [128, G, OC]
                            ),
                            op=ALU.mult,
                        )
                        nc.vector.tensor_copy(out=cCb[:], in_=cC[:])
                        # call[p,(g,b',o)] = cCb[p,(g,o)] * delta(b(p)==b')
                        nc.vector.tensor_tensor(
                            out=call[:].rearrange(
                                "p (g b o) -> p g b o", b=BG, o=OC
                            ),
                            in0=cCb[:]
                            .rearrange("p (g o) -> p g o", o=OC)
                            .unsqueeze(2)
                            .broadcast_to([128, G, BG, OC]),
                            in1=mcb[:]
                            .rearrange("p (b o) -> p b o", o=OC)
                            .unsqueeze(1)
                            .broadcast_to([128, G, BG, OC]),
                            op=ALU.mult,
                        )

                    # ---- s matmul: accumulate over all 72 groups
                    ps = ps_pool.tile([80, ODF], dt.float32, tag="ps")
                    for g in range(G):
                        lhsT = (
                            c0t[:]
                            if it == 0
                            else call[:, g * 80 : (g + 1) * 80]
                        )
                        nc.tensor.matmul(
                            ps[:],
                            lhsT,
                            uh[:, g * ODF : (g + 1) * ODF],
                            start=(g == 0),
                            stop=(g == G - 1),
                        )

                    if LEVEL == 20:
                        dbg20 = spool.tile([80, OD], dt.float32, tag="dbg20")
                        nc.vector.tensor_copy(out=dbg20[:], in_=ps[:80, :OD])
                        nc.sync.dma_start(
                            out=vout_d[bg * BG : (bg + 1) * BG].rearrange(
                                "b o d -> (b o) d"
                            ),
                            in_=dbg20[:],
                        )
                        continue
                    # ---- extract diag o==o' -> s [80,16]; transpose free to
                    # (d, o') so the o' reduce is innermost
                    tmp = spool.tile([80, ODF], dt.float32, tag="tmp")
                    nc.vector.tensor_tensor(
                        out=tmp[:].rearrange("p (d o) -> p d o", o=OC),
                        in0=ps[:]
                        .rearrange("p (o d) -> p o d", o=OC)
                        .transpose([0, 2, 1]),
                        in1=msks[:]
                        .rearrange("p (o d) -> p o d", o=OC)
                        .transpose([0, 2, 1]),
                        op=ALU.mult,
                    )
                    s_t = spool.tile([80, OD], dt.float32, tag="s_t")
                    nc.vector.tensor_reduce(
                        out=s_t[:],
                        in_=tmp[:].rearrange("p (d o) -> p d o", o=OC),
                        axis=AX.X,
                        op=ALU.add,
                    )

                    if LEVEL == 21:
                        nc.sync.dma_start(
                            out=vout_d[bg * BG : (bg + 1) * BG].rearrange(
                                "b o d -> (b o) d"
                            ),
                            in_=s_t[:],
                        )
                        continue
                    # ---- squash
                    sq = spool.tile([80, OD], dt.float32, tag="sq")
                    ns = spool.tile([80, 1], dt.float32, tag="ns")
                    nc.scalar.activation(
                        out=sq[:], in_=s_t[:], func=AF.Square, bias=czero[:80]
                    )
                    nc.vector.tensor_reduce(
                        out=ns[:], in_=sq[:], axis=AX.X, op=ALU.add
                    )
                    if LEVEL == 23:
                        nc.sync.dma_start(
                            out=vout_d[bg * BG : (bg + 1) * BG].rearrange(
                                "b o d -> (b o) d"
                            ),
                            in_=sq[:],
                        )
                        continue
                    # fac = ns / ((1+ns) * sqrt(ns+eps))
                    sqn = spool.tile([80, 1], dt.float32, tag="sqn")
                    nc.scalar.activation(
                        out=sqn[:], in_=ns[:], func=AF.Sqrt, bias=ceps[:]
                    )
                    den = spool.tile([80, 1], dt.float32, tag="den")
                    nc.vector.scalar_tensor_tensor(
                        out=den[:],
                        in0=ns[:],
                        scalar=1.0,
                        in1=sqn[:],
                        op0=ALU.add,
                        op1=ALU.mult,
                    )
                    if LEVEL == 24:
                        nc.sync.dma_start(
                            out=vout_d[bg * BG : (bg + 1) * BG].rearrange(
                                "b o d -> (b o) d"
                            ),
                            in_=sq[:],
                        )
                        continue
                    rden = spool.tile([80, 1], dt.float32, tag="rden")
                    nc.vector.reciprocal(out=rden[:], in_=den[:])
                    fac = spool.tile([80, 1], dt.float32, tag="fac")
                    nc.vector.tensor_tensor(
                        out=fac[:], in0=ns[:], in1=rden[:], op=ALU.mult
                    )

                    if LEVEL == 22:
                        nc.sync.dma_start(
                            out=vout_d[bg * BG : (bg + 1) * BG].rearrange(
                                "b o d -> (b o) d"
                            ),
                            in_=sq[:],
                        )
                        continue
                    if it == n_it - 1 and LEVEL in (2, 5):
                        v_f = spool.tile([80, OD], dt.float32, tag="v_f")
                        nc.vector.tensor_scalar_mul(v_f[:], s_t[:], fac[:])
                        nc.sync.dma_start(
                            out=vout_d[bg * BG : (bg + 1) * BG].rearrange(
                                "b o d -> (b o) d"
                            ),
                            in_=v_f[:],
                        )
                        continue

                    if LEVEL < 3:
                        continue
                    v_bf = spool.tile([80, OD], dt.bfloat16, tag="v_bf")
                    nc.vector.tensor_scalar_mul(v_bf[:], s_t[:], fac[:])

                    # ---- vexp[(b,o),(o',d)] = v[b,o,d] * delta(o==o')
                    vexp = spool.tile([80, ODF], dt.bfloat16, tag="vexp")
                    nc.vector.tensor_tensor(
                        out=vexp[:].rearrange("p (o d) -> p o d", o=OC),
                        in0=msks[:].rearrange("p (o d) -> p o d", o=OC),
                        in1=v_bf[:]
                        .unsqueeze(1)
                        .broadcast_to([80, OC, OD]),
                        op=ALU.mult,
                    )
                    # vrep[(i_sub,b'),(o,d)] = v[b',o,d] via arep matmul
                    pv = pv_pool.tile([128, ODF], dt.float32, tag="pv")
                    nc.tensor.matmul(pv[:], arep[:], vexp[:], start=True, stop=True)
                    vrep = spool.tile([128, ODF], dt.bfloat16, tag="vrep")
                    nc.scalar.copy(out=vrep[:], in_=pv[:])
                    if LEVEL == 3 and it == n_it - 1:
                        dbg3 = spool.tile([80, OD], dt.float32, tag="dbg3")
                        nc.vector.tensor_copy(out=dbg3[:], in_=vrep[:80, :OD])
                        nc.sync.dma_start(
                            out=vout_d[bg * BG : (bg + 1) * BG].rearrange(
                                "b o d -> (b o) d"
                            ),
                            in_=dbg3[:],
                        )
                    if LEVEL < 4:
                        continue

                    # ---- bl += sum_d u_hat * vrep, chunked over g
                    for ch in range(NZCH):
                        cs = ch * ZCH
                        z = zpool.tile([128, ZCH * ODF], dt.bfloat16, tag="z")
                        nc.vector.tensor_tensor(
                            out=z[:].rearrange("p (g f) -> p g f", f=ODF),
                            in0=uh[
                                :, cs * ODF : (cs + ZCH) * ODF
                            ].rearrange("p (g f) -> p g f", f=ODF),
                            in1=vrep[:]
                            .unsqueeze(1)
                            .broadcast_to([128, ZCH, ODF]),
                            op=ALU.mult,
                        )
                        t8 = zpool.tile([128, ZCH * 80], dt.bfloat16, tag="t8")
                        zv = z[:].rearrange(
                            "p (g o d) -> p g o d", o=OC, d=OD
                        )
                        nc.vector.tensor_tensor(
                            out=t8[:].rearrange(
                                "p (g o d) -> p g o d", o=OC, d=8
                            ),
                            in0=zv[:, :, :, 0:8],
                            in1=zv[:, :, :, 8:16],
                            op=ALU.add,
                        )
                        t4 = zpool.tile([128, ZCH * 40], dt.bfloat16, tag="t4")
                        t8v = t8[:].rearrange(
                            "p (g o d) -> p g o d", o=OC, d=8
                        )
                        nc.vector.tensor_tensor(
                            out=t4[:].rearrange(
                                "p (g o d) -> p g o d", o=OC, d=4
                            ),
                            in0=t8v[:, :, :, 0:4],
                            in1=t8v[:, :, :, 4:8],
                            op=ALU.add,
                        )
                        t2 = zpool.tile([128, ZCH * 20], dt.bfloat16, tag="t2")
                        t4v = t4[:].rearrange(
                            "p (g o d) -> p g o d", o=OC, d=4
                        )
                        nc.vector.tensor_tensor(
                            out=t2[:].rearrange(
                                "p (g o d) -> p g o d", o=OC, d=2
                            ),
                            in0=t4v[:, :, :, 0:2],
                            in1=t4v[:, :, :, 2:4],
                            op=ALU.add,
                        )
                        t1 = zpool.tile([128, ZCH * OC], dt.float32, tag="t1")
                        t2v = t2[:].rearrange(
                            "p (g o d) -> p g o d", o=OC, d=2
                        )
                        nc.vector.tensor_tensor(
                            out=t1[:].rearrange("p (g o) -> p g o", o=OC).unsqueeze(3),
                            in0=t2v[:, :, :, 0:1],
                            in1=t2v[:, :, :, 1:2],
                            op=ALU.add,
                        )
                        nc.vector.tensor_tensor(
                            out=bl[:, cs * OC : (cs + ZCH) * OC],
                            in0=bl[:, cs * OC : (cs + ZCH) * OC],
                            in1=t1[:],
                            op=ALU.add,
                        )
                    if LEVEL == 4 and it == n_it - 1:
                        dbg4 = spool.tile([80, OD], dt.float32, tag="dbg4")
                        nc.vector.tensor_copy(out=dbg4[:], in_=bl[:80, :OD])
                        nc.sync.dma_start(
                            out=vout_d[bg * BG : (bg + 1) * BG].rearrange(
                                "b o d -> (b o) d"
                            ),
                            in_=dbg4[:],
                        )

    nc.finalize()
    _BUILT = nc
    return nc


def kernel(x, W):
    x = np.asarray(x, np.float32)
    W = np.asarray(W, np.float32)
    nc = _build()
    mcb, c0t, msks, arep = _consts()
    wr = _prep_w(W)
    in_maps = []
    for c in range(NCORES):
        m = _prep_core(x[c * BL : (c + 1) * BL], wr)
        m.update(wr=wr, mcb=mcb, c0t=c0t, msks=msks, arep=arep)
        in_maps.append(m)
    res = run_bass_kernel_spmd(nc, in_maps, core_ids=list(range(NCORES)))
    outs = res.results
    v = np.concatenate([np.asarray(o["vout"]) for o in outs], axis=0)
    return v.astype(np.float32)


if __name__ == "__main__":
    rng = np.random.default_rng(0)
    x = rng.standard_normal((B, IC, KD), np.float32)
    W = rng.standard_normal((IC, OC, OD, KD), np.float32)
    v = kernel(x, W)
    print("out", v.shape, v.dtype, float(np.abs(v).mean()))
